# revision 33
# baseline (speedup 1.0000x reference)
"""DepLabeledGCN Trainium2 kernel — data-parallel, DMA-schedule-optimized.

Each core processes ITS OWN batch with ALL 48 label matrices (B=8 == 8
cores, no collectives).  Per layer:
    s-phase:  sT[l,kc] = per-label masked-adjacency matmuls; masks are
              exact 0/1 stored FP8 (halves mask SBUF), label pairs fused
              into 256-wide matmuls
    msum:     msg = sum_{l,kc} sT[l,kc] @ W_l^T[kc], 192 accumulating
              matmuls into one PSUM bank
    relu(msg * 1/denom) -> next layer h, split across vector+scalar

Scheduling (measured path from the 155 us v1 baseline to ~144 us):
  * Layer 1 is DMA-bound: all 24 MB of fp16 weights must land during it
    (layer 2 then runs at the PE roofline, ~54 us).  R_RES=36 labels
    stay resident (masks fp8 + gcn pre-cast fp16 free the SBUF), so 12
    labels re-stream for layer 2; one shared 7-buf stream pool lets the
    re-fetch DMAs ride the tail of layer 1 / idle layer-2 window.
  * Resident weights load as 1 MB PAIR descriptors: the sync engine
    needs ~0.6 us to issue each DMA, so 512 KB descriptors cap
    concurrency (~330 GB/s); pairs sustain the ring at ~420 GB/s.
  * The sT PSUM->SBUF cast is split in half across vector+scalar every
    pair: its completion gates the s-phase PSUM buffer recycle, and a
    single 1.2 us cast stalled the PE ~0.8 us per pair in both layers.
  * adjT/labT/adjR ride in one packed input descriptor ahead of the
    weight stream; gcn arrives as fp16 (h0 directly, no cast).
  * MLP: grouped transposes (PE order = emission order, no PE<->DVE
    ping-pong), per-chunk copies, 512-wide moving matmuls, bias via a
    ones-row matmul opening the PSUM group, output [N, D]-major with
    one 256 KB DMA.
"""

import sys

if '/opt/trn_rl_repo' not in sys.path:
    sys.path.insert(0, '/opt/trn_rl_repo')

import numpy as np

B, N, D, L = 8, 128, 512, 48
NCORES = 8
KC = D // 128
NUM_LAYERS = 2
K8 = 20                 # labels whose LAYER-1 weights are fp8 (x64 scale)
R16 = 24                # fp16-resident labels (K8..K8+R16-1)
R_RES = K8 + R16        # labels needing no layer-2 re-stream
NP = L // 2             # label pairs per layer

_CACHE = {}


def _build_nc():
    import concourse.bass as bass
    import concourse.mybir as mybir
    import concourse.tile as tile
    from concourse import bacc
    from concourse.masks import make_identity

    dt = mybir.dt
    f32 = dt.float32
    f16 = dt.float16
    f8 = dt.float8e4
    Alu = mybir.AluOpType
    Act = mybir.ActivationFunctionType

    nc = bacc.Bacc("TRN2", target_bir_lowering=False, debug=False,
                   num_devices=NCORES)

    gcn16_e = nc.dram_tensor("gcn16", [N, D], f16, kind="ExternalInput").ap()
    alr_e = nc.dram_tensor("alr", [N, 3, N], f32, kind="ExternalInput").ap()
    wT_e = nc.dram_tensor("wT", [128, L, KC, D], f16, kind="ExternalInput").ap()
    wT8_e = nc.dram_tensor("wT8", [128, K8, KC, D], f8, kind="ExternalInput").ap()
    w0T_e = nc.dram_tensor("w0T", [128, KC, D], f16, kind="ExternalInput").ap()
    w1T_e = nc.dram_tensor("w1T", [128, KC, D], f16, kind="ExternalInput").ap()
    b0_e = nc.dram_tensor("b0", [1, D], f16, kind="ExternalInput").ap()
    b1_e = nc.dram_tensor("b1", [1, D], f16, kind="ExternalInput").ap()
    out_e = nc.dram_tensor("out", [N, D], f32, kind="ExternalOutput").ap()

    with tile.TileContext(nc) as tc:
        with (
            tc.tile_pool(name="const", bufs=1) as cpool,
            tc.tile_pool(name="sT", bufs=4) as sT_pool,
            tc.tile_pool(name="wst", bufs=8) as wst_pool,
            tc.tile_pool(name="spsum", bufs=3, space="PSUM") as spsum,
            tc.tile_pool(name="mpsum", bufs=2, space="PSUM") as mpsum,
        ):
            # -------- critical-path input loads -----------------------------
            alr_sb = cpool.tile([128, 3, N], f32, tag="alr")
            nc.sync.dma_start(alr_sb[:], alr_e)
            adjT_sb = alr_sb[:, 0, :]
            labT_sb = alr_sb[:, 1, :]
            adjR_sb = alr_sb[:, 2, :]
            h = [cpool.tile([128, D], f16, tag=f"h{ly}", name=f"h{ly}")
                 for ly in range(NUM_LAYERS + 1)]
            nc.sync.dma_start(h[0][:], gcn16_e)

            # resident weights: labels 0..K8-1 are fp8 (x64) for layer 1
            # (halved bytes on layer-1's critical DMA path); labels
            # K8..K8+R16-1 are fp16, shared by both layers
            wres8 = cpool.tile([128, K8, KC, D], f8, tag="wres8")
            nc.sync.dma_start(wres8[:, 0:2], wT8_e[:, 0:2])
            for l in range(2, K8, 4):
                le = min(l + 4, K8)
                nc.sync.dma_start(wres8[:, l:le], wT8_e[:, l:le])
            wres = cpool.tile([128, R16, KC, D], f16, tag="wres")
            for l in range(K8, R_RES, 2):
                nc.sync.dma_start(wres[:, l - K8:l - K8 + 2],
                                  wT_e[:, l:l + 2])

            # -------- masks: maskT[j, l, i] = (labT == l) * adjT ------------
            # exact 0/1 values -> fp8 is lossless and halves SBUF
            maskT = cpool.tile([128, L, N], f8, tag="maskT")

            def emit_mask(l):
                nc.vector.scalar_tensor_tensor(
                    out=maskT[:, l, :],
                    in0=labT_sb,
                    scalar=float(l),
                    in1=adjT_sb,
                    op0=Alu.is_equal,
                    op1=Alu.mult,
                )

            for l in range(12):
                emit_mask(l)

            den = cpool.tile([128, 1], f32, tag="den")
            nc.vector.tensor_reduce(den[:], adjR_sb, mybir.AxisListType.X,
                                    Alu.add)
            nc.vector.tensor_scalar_add(den[:], den[:], 1.0)
            recip = cpool.tile([128, 1], f32, tag="recip")
            nc.vector.reciprocal(recip[:], den[:])

            # -------- GCN layers --------------------------------------------
            def emit_s(ly, p):
                """s-phase for label pair p: one N=256 matmul per kc."""
                ps = spsum.tile([128, KC, 2, 128], f32, tag="spsum",
                                name="spsum")
                for kc in range(KC):
                    nc.tensor.matmul(
                        ps[:, kc, :, :],
                        lhsT=h[ly][:, kc * 128:(kc + 1) * 128],
                        rhs=maskT[:, 2 * p:2 * p + 2, :],
                        start=True, stop=True,
                    )
                # PSUM->SBUF cast split across both engines: halves the
                # latency that gates the spsum buffer recycle.  Pairs of
                # fp8 labels get s/64 (weights are stored x64 in both
                # the fp8 plane and their fp16 layer-2 copies).
                sT = sT_pool.tile([128, KC, 2, 128], f16, tag="sT", name="sT")
                if 2 * p < K8:
                    nc.vector.tensor_scalar(sT[:, 0:2], ps[:, 0:2],
                                            1.0 / 64.0, None, Alu.mult)
                    nc.scalar.activation(sT[:, 2:4], ps[:, 2:4], Act.Copy,
                                         scale=1.0 / 64.0)
                else:
                    nc.vector.tensor_copy(sT[:, 0:2], ps[:, 0:2])
                    nc.scalar.copy(sT[:, 2:4], ps[:, 2:4])
                return sT

            def get_w(ly, l):
                """Weight for label l: fp8 plane (layer 1), fp16
                resident slice, or streamed fp16 tile."""
                if l < K8:
                    if ly == 0:
                        return wres8[:, l]
                elif l < R_RES:
                    return wres[:, l - K8]
                w = wst_pool.tile([128, KC, D], f16, tag="wst", name="wst")
                nc.sync.dma_start(w[:], wT_e[:, l])
                return w

            P8 = K8 // 2
            PR = R_RES // 2
            # layer-2 order: two resident pairs lead (DMA head start),
            # then the streamed-fp16 copies of the fp8 labels -- consuming
            # them early releases the stream pool's buffers, so the
            # in-order DMA ring never head-of-line blocks on a WAR
            # trigger; the tail re-fetches (42-47) then land ~35 us
            # before their use
            orders = [list(range(NP)),
                      [P8, P8 + 1, P8 + 2, P8 + 3] + list(range(P8))
                      + list(range(P8 + 4, PR)) + list(range(PR, NP))]

            for ly in range(NUM_LAYERS):
                order = orders[ly]
                pm = mpsum.tile([128, D], f32, tag="mm", name="mm")
                sT_q = {order[0]: emit_s(ly, order[0]),
                        order[1]: emit_s(ly, order[1])}
                for pi in range(NP):
                    p = order[pi]
                    if ly == 0 and 2 * (pi + 6) < L:
                        emit_mask(2 * (pi + 6))
                        emit_mask(2 * (pi + 6) + 1)
                    if pi + 2 < NP:
                        pn = order[pi + 2]
                        sT_q[pn] = emit_s(ly, pn)
                    sT = sT_q.pop(p)
                    for l2 in range(2):
                        l = 2 * p + l2
                        w = get_w(ly, l)
                        for kc in range(KC):
                            i = pi * 2 * KC + l2 * KC + kc
                            nc.tensor.matmul(
                                pm[:],
                                lhsT=sT[:, kc, l2, :],
                                rhs=w[:, kc, :],
                                start=(i == 0), stop=(i == L * KC - 1),
                            )
                # relu(msg * recip) -> next h (fp16), halves on both engines
                hn = h[ly + 1]
                nc.vector.tensor_scalar(hn[:, 0:256], pm[:, 0:256],
                                        recip[:], 0.0, Alu.mult, Alu.max)
                nc.scalar.activation(hn[:, 256:512], pm[:, 256:512],
                                     Act.Relu, scale=recip[:])

            # -------- MLP ---------------------------------------------------
            # x = relu(x @ w^T + b), wide form: transposed activation is
            # stationary, w^T is 512-wide moving, bias via ones-row matmul.
            w0T_sb = cpool.tile([128, KC, D], f16, tag="w0T")
            nc.sync.dma_start(w0T_sb[:], w0T_e)
            w1T_sb = cpool.tile([128, KC, D], f16, tag="w1T")
            nc.sync.dma_start(w1T_sb[:], w1T_e)
            b0_sb = cpool.tile([1, D], f16, tag="b0")
            nc.sync.dma_start(b0_sb[:], b0_e)
            b1_sb = cpool.tile([1, D], f16, tag="b1")
            nc.sync.dma_start(b1_sb[:], b1_e)
            ones = cpool.tile([1, 128], f16, tag="ones")
            nc.gpsimd.memset(ones[:], 1.0)
            identity = cpool.tile([128, 128], f16, tag="ident")
            make_identity(nc, identity[:])

            xT = cpool.tile([128, KC, 128], f16, tag="xT")

            def mlp_layer(x_in, w_sb, b_sb, out_tile, name):
                # grouped emission (PE order = emission order: no PE->DVE
                # ping-pong); per-chunk copies so mm_kc starts after
                # copy_kc; bias matmul opens the PSUM group
                pt = mpsum.tile([128, KC, 128], f16, tag="mm", name=name)
                px = mpsum.tile([128, D], f32, tag="mm", name=name + "p")
                nc.tensor.matmul(px[:], lhsT=ones[:], rhs=b_sb[:],
                                 start=True, stop=False)
                for kc in range(KC):
                    nc.tensor.transpose(pt[:, kc, :],
                                        x_in[:, kc * 128:(kc + 1) * 128],
                                        identity[:])
                for kc in range(KC):
                    eng = nc.vector.tensor_copy if kc % 2 == 0 \
                        else nc.scalar.copy
                    eng(xT[:, kc], pt[:, kc, :])
                for kc in range(KC):
                    nc.tensor.matmul(px[:], lhsT=xT[:, kc], rhs=w_sb[:, kc],
                                     start=False, stop=(kc == KC - 1))
                nc.vector.tensor_scalar(out_tile[:, 0:256], px[:, 0:256],
                                        0.0, None, Alu.max)
                nc.scalar.activation(out_tile[:, 256:512], px[:, 256:512],
                                     Act.Relu)

            x1 = cpool.tile([128, D], f16, tag="x1")
            mlp_layer(h[NUM_LAYERS], w0T_sb, b0_sb, x1, "m1")
            x2 = cpool.tile([128, D], f32, tag="x2")
            mlp_layer(x1, w1T_sb, b1_sb, x2, "m2")

            nc.sync.dma_start(out_e[:, 0:256], x2[:, 0:256])
            nc.sync.dma_start(out_e[:, 256:512], x2[:, 256:512])

    nc.compile()
    return nc


def _get_nc():
    if "nc" not in _CACHE:
        _CACHE["nc"] = _build_nc()
    return _CACHE["nc"]


def kernel(gcn_inputs, word_seq_len, adj_matrix, dep_label_matrix,
           w_params, mlp_w0, mlp_b0, mlp_w1, mlp_b1, **_unused):
    from concourse.bass_utils import run_bass_kernel_spmd

    gcn = np.asarray(gcn_inputs, dtype=np.float32)
    adj = np.asarray(adj_matrix, dtype=np.float32)
    lab = np.asarray(dep_label_matrix)
    w = np.asarray(w_params, dtype=np.float32)
    w0 = np.asarray(mlp_w0, dtype=np.float32)
    w1 = np.asarray(mlp_w1, dtype=np.float32)
    b0 = np.asarray(mlp_b0, dtype=np.float32)
    b1 = np.asarray(mlp_b1, dtype=np.float32)

    import ml_dtypes

    # wT[kmod, l, kc, d] = w[l, d, kc*128+kmod]  (shared by all cores).
    # Labels < K8 are stored x64 in BOTH planes (fp8 for layer 1, fp16
    # for layer 2 -- a lossless exponent shift); the kernel scales their
    # aggregated s by 1/64 instead, keeping the fp8 values in e4m3's
    # normal range.
    ws = w.copy()
    ws[:K8] *= 64.0
    wT = ws.transpose(0, 2, 1).reshape(L, KC, 128, D).transpose(2, 0, 1, 3)
    wT = np.ascontiguousarray(wT)
    wT8 = wT[:, :K8].astype(ml_dtypes.float8_e4m3)
    wT = wT.astype(np.float16)
    w0T = np.ascontiguousarray(
        w0.T.reshape(KC, 128, D).transpose(1, 0, 2)).astype(np.float16)
    w1T = np.ascontiguousarray(
        w1.T.reshape(KC, 128, D).transpose(1, 0, 2)).astype(np.float16)
    b0r = b0.reshape(1, D).astype(np.float16)
    b1r = b1.reshape(1, D).astype(np.float16)
    labf = lab.astype(np.float32)

    in_maps = []
    for c in range(NCORES):
        alr = np.stack([adj[c].T, labf[c].T, adj[c]], axis=1)
        in_maps.append({
            "gcn16": gcn[c].astype(np.float16),
            "alr": np.ascontiguousarray(alr),
            "wT": wT,
            "wT8": wT8,
            "w0T": w0T,
            "w1T": w1T,
            "b0": b0r,
            "b1": b1r,
        })

    nc = _get_nc()
    res = run_bass_kernel_spmd(nc, in_maps, list(range(NCORES)))

    out = np.empty((B, N, D), dtype=np.float32)
    for c in range(NCORES):
        out[c] = res.results[c]["out"]
    return out


# revision 34
# speedup vs baseline: 1.0526x; 1.0526x over previous
"""DepLabeledGCN Trainium2 kernel — data-parallel, DMA-schedule-optimized.

Each core processes ITS OWN batch with ALL 48 label matrices (B=8 == 8
cores, no collectives).  Per layer:
    s-phase:  sT[l,kc] = per-label masked-adjacency matmuls; masks are
              exact 0/1 stored FP8 (halves mask SBUF), label pairs fused
              into 256-wide matmuls
    msum:     msg = sum_{l,kc} sT[l,kc] @ W_l^T[kc], 192 accumulating
              matmuls into one PSUM bank
    relu(msg * 1/denom) -> next layer h, split across vector+scalar

Scheduling (measured path from the 155 us v1 baseline to ~144 us):
  * Layer 1 is DMA-bound: all 24 MB of fp16 weights must land during it
    (layer 2 then runs at the PE roofline, ~54 us).  R_RES=36 labels
    stay resident (masks fp8 + gcn pre-cast fp16 free the SBUF), so 12
    labels re-stream for layer 2; one shared 7-buf stream pool lets the
    re-fetch DMAs ride the tail of layer 1 / idle layer-2 window.
  * Resident weights load as 1 MB PAIR descriptors: the sync engine
    needs ~0.6 us to issue each DMA, so 512 KB descriptors cap
    concurrency (~330 GB/s); pairs sustain the ring at ~420 GB/s.
  * The sT PSUM->SBUF cast is split in half across vector+scalar every
    pair: its completion gates the s-phase PSUM buffer recycle, and a
    single 1.2 us cast stalled the PE ~0.8 us per pair in both layers.
  * adjT/labT/adjR ride in one packed input descriptor ahead of the
    weight stream; gcn arrives as fp16 (h0 directly, no cast).
  * MLP: grouped transposes (PE order = emission order, no PE<->DVE
    ping-pong), per-chunk copies, 512-wide moving matmuls, bias via a
    ones-row matmul opening the PSUM group, output [N, D]-major with
    one 256 KB DMA.
"""

import sys

if '/opt/trn_rl_repo' not in sys.path:
    sys.path.insert(0, '/opt/trn_rl_repo')

import numpy as np

B, N, D, L = 8, 128, 512, 48
NCORES = 8
KC = D // 128
NUM_LAYERS = 2
K8 = 24                 # labels whose LAYER-1 weights are fp8 (x64 scale)
R16 = 20                # fp16-resident labels (K8..K8+R16-1)
R_RES = K8 + R16        # labels needing no layer-2 re-stream
NP = L // 2             # label pairs per layer

_CACHE = {}


def _build_nc():
    import concourse.bass as bass
    import concourse.mybir as mybir
    import concourse.tile as tile
    from concourse import bacc
    from concourse.masks import make_identity

    dt = mybir.dt
    f32 = dt.float32
    f16 = dt.float16
    f8 = dt.float8e4
    Alu = mybir.AluOpType
    Act = mybir.ActivationFunctionType

    nc = bacc.Bacc("TRN2", target_bir_lowering=False, debug=False,
                   num_devices=NCORES)

    gcn16_e = nc.dram_tensor("gcn16", [N, D], f16, kind="ExternalInput").ap()
    alr_e = nc.dram_tensor("alr", [N, 3, N], f32, kind="ExternalInput").ap()
    wT_e = nc.dram_tensor("wT", [128, L, KC, D], f16, kind="ExternalInput").ap()
    wT8_e = nc.dram_tensor("wT8", [128, K8, KC, D], f8, kind="ExternalInput").ap()
    w0T_e = nc.dram_tensor("w0T", [128, KC, D], f16, kind="ExternalInput").ap()
    w1T_e = nc.dram_tensor("w1T", [128, KC, D], f16, kind="ExternalInput").ap()
    b0_e = nc.dram_tensor("b0", [1, D], f16, kind="ExternalInput").ap()
    b1_e = nc.dram_tensor("b1", [1, D], f16, kind="ExternalInput").ap()
    out_e = nc.dram_tensor("out", [N, D], f32, kind="ExternalOutput").ap()

    with tile.TileContext(nc) as tc:
        with (
            tc.tile_pool(name="const", bufs=1) as cpool,
            tc.tile_pool(name="sT", bufs=4) as sT_pool,
            tc.tile_pool(name="wst", bufs=10) as wst_pool,
            tc.tile_pool(name="spsum", bufs=3, space="PSUM") as spsum,
            tc.tile_pool(name="mpsum", bufs=2, space="PSUM") as mpsum,
        ):
            # -------- critical-path input loads -----------------------------
            alr_sb = cpool.tile([128, 3, N], f32, tag="alr")
            nc.sync.dma_start(alr_sb[:], alr_e)
            adjT_sb = alr_sb[:, 0, :]
            labT_sb = alr_sb[:, 1, :]
            adjR_sb = alr_sb[:, 2, :]
            h = [cpool.tile([128, D], f16, tag=f"h{ly}", name=f"h{ly}")
                 for ly in range(NUM_LAYERS + 1)]
            nc.sync.dma_start(h[0][:], gcn16_e)

            # resident weights: labels 0..K8-1 are fp8 (x64) for layer 1
            # (halved bytes on layer-1's critical DMA path); labels
            # K8..K8+R16-1 are fp16, shared by both layers
            wres8 = cpool.tile([128, K8, KC, D], f8, tag="wres8")
            nc.sync.dma_start(wres8[:, 0:2], wT8_e[:, 0:2])
            for l in range(2, K8, 4):
                le = min(l + 4, K8)
                nc.sync.dma_start(wres8[:, l:le], wT8_e[:, l:le])
            wres = cpool.tile([128, R16, KC, D], f16, tag="wres")
            for l in range(K8, R_RES, 2):
                nc.sync.dma_start(wres[:, l - K8:l - K8 + 2],
                                  wT_e[:, l:l + 2])

            # -------- masks: maskT[j, l, i] = (labT == l) * adjT ------------
            # exact 0/1 values -> fp8 is lossless and halves SBUF
            maskT = cpool.tile([128, L, N], f8, tag="maskT")

            def emit_mask(l):
                nc.vector.scalar_tensor_tensor(
                    out=maskT[:, l, :],
                    in0=labT_sb,
                    scalar=float(l),
                    in1=adjT_sb,
                    op0=Alu.is_equal,
                    op1=Alu.mult,
                )

            for l in range(12):
                emit_mask(l)

            den = cpool.tile([128, 1], f32, tag="den")
            nc.vector.tensor_reduce(den[:], adjR_sb, mybir.AxisListType.X,
                                    Alu.add)
            nc.vector.tensor_scalar_add(den[:], den[:], 1.0)
            recip = cpool.tile([128, 1], f32, tag="recip")
            nc.vector.reciprocal(recip[:], den[:])

            # -------- GCN layers --------------------------------------------
            def emit_s(ly, p):
                """s-phase for label pair p: one N=256 matmul per kc."""
                ps = spsum.tile([128, KC, 2, 128], f32, tag="spsum",
                                name="spsum")
                for kc in range(KC):
                    nc.tensor.matmul(
                        ps[:, kc, :, :],
                        lhsT=h[ly][:, kc * 128:(kc + 1) * 128],
                        rhs=maskT[:, 2 * p:2 * p + 2, :],
                        start=True, stop=True,
                    )
                # PSUM->SBUF cast split across both engines: halves the
                # latency that gates the spsum buffer recycle.  Pairs of
                # fp8 labels get s/64 (weights are stored x64 in both
                # the fp8 plane and their fp16 layer-2 copies).
                sT = sT_pool.tile([128, KC, 2, 128], f16, tag="sT", name="sT")
                if 2 * p < K8:
                    nc.vector.tensor_scalar(sT[:, 0:2], ps[:, 0:2],
                                            1.0 / 64.0, None, Alu.mult)
                    nc.scalar.activation(sT[:, 2:4], ps[:, 2:4], Act.Copy,
                                         scale=1.0 / 64.0)
                else:
                    nc.vector.tensor_copy(sT[:, 0:2], ps[:, 0:2])
                    nc.scalar.copy(sT[:, 2:4], ps[:, 2:4])
                return sT

            def get_w(ly, l):
                """Weight for label l: fp8 plane (layer 1), fp16
                resident slice, or streamed fp16 tile."""
                if l < K8:
                    if ly == 0:
                        return wres8[:, l]
                elif l < R_RES:
                    return wres[:, l - K8]
                w = wst_pool.tile([128, KC, D], f16, tag="wst", name="wst")
                nc.sync.dma_start(w[:], wT_e[:, l])
                return w

            P8 = K8 // 2
            PR = R_RES // 2
            # layer-2 order: two resident pairs lead (DMA head start),
            # then the streamed-fp16 copies of the fp8 labels -- consuming
            # them early releases the stream pool's buffers, so the
            # in-order DMA ring never head-of-line blocks on a WAR
            # trigger; the tail re-fetches (42-47) then land ~35 us
            # before their use
            orders = [list(range(NP)),
                      [P8, P8 + 1, P8 + 2, P8 + 3] + list(range(P8))
                      + list(range(P8 + 4, PR)) + list(range(PR, NP))]

            for ly in range(NUM_LAYERS):
                order = orders[ly]
                pm = mpsum.tile([128, D], f32, tag="mm", name="mm")
                sT_q = {order[0]: emit_s(ly, order[0]),
                        order[1]: emit_s(ly, order[1])}
                for pi in range(NP):
                    p = order[pi]
                    if ly == 0 and 2 * (pi + 6) < L:
                        emit_mask(2 * (pi + 6))
                        emit_mask(2 * (pi + 6) + 1)
                    if pi + 2 < NP:
                        pn = order[pi + 2]
                        sT_q[pn] = emit_s(ly, pn)
                    sT = sT_q.pop(p)
                    for l2 in range(2):
                        l = 2 * p + l2
                        w = get_w(ly, l)
                        for kc in range(KC):
                            i = pi * 2 * KC + l2 * KC + kc
                            nc.tensor.matmul(
                                pm[:],
                                lhsT=sT[:, kc, l2, :],
                                rhs=w[:, kc, :],
                                start=(i == 0), stop=(i == L * KC - 1),
                            )
                # relu(msg * recip) -> next h (fp16), halves on both engines
                hn = h[ly + 1]
                nc.vector.tensor_scalar(hn[:, 0:256], pm[:, 0:256],
                                        recip[:], 0.0, Alu.mult, Alu.max)
                nc.scalar.activation(hn[:, 256:512], pm[:, 256:512],
                                     Act.Relu, scale=recip[:])

            # -------- MLP ---------------------------------------------------
            # x = relu(x @ w^T + b), wide form: transposed activation is
            # stationary, w^T is 512-wide moving, bias via ones-row matmul.
            w0T_sb = cpool.tile([128, KC, D], f16, tag="w0T")
            nc.sync.dma_start(w0T_sb[:], w0T_e)
            w1T_sb = cpool.tile([128, KC, D], f16, tag="w1T")
            nc.sync.dma_start(w1T_sb[:], w1T_e)
            b0_sb = cpool.tile([1, D], f16, tag="b0")
            nc.sync.dma_start(b0_sb[:], b0_e)
            b1_sb = cpool.tile([1, D], f16, tag="b1")
            nc.sync.dma_start(b1_sb[:], b1_e)
            ones = cpool.tile([1, 128], f16, tag="ones")
            nc.gpsimd.memset(ones[:], 1.0)
            identity = cpool.tile([128, 128], f16, tag="ident")
            make_identity(nc, identity[:])

            xT = cpool.tile([128, KC, 128], f16, tag="xT")

            def mlp_layer(x_in, w_sb, b_sb, out_tile, name):
                # grouped emission (PE order = emission order: no PE->DVE
                # ping-pong); per-chunk copies so mm_kc starts after
                # copy_kc; bias matmul opens the PSUM group
                pt = mpsum.tile([128, KC, 128], f16, tag="mm", name=name)
                px = mpsum.tile([128, D], f32, tag="mm", name=name + "p")
                nc.tensor.matmul(px[:], lhsT=ones[:], rhs=b_sb[:],
                                 start=True, stop=False)
                for kc in range(KC):
                    nc.tensor.transpose(pt[:, kc, :],
                                        x_in[:, kc * 128:(kc + 1) * 128],
                                        identity[:])
                for kc in range(KC):
                    eng = nc.vector.tensor_copy if kc % 2 == 0 \
                        else nc.scalar.copy
                    eng(xT[:, kc], pt[:, kc, :])
                for kc in range(KC):
                    nc.tensor.matmul(px[:], lhsT=xT[:, kc], rhs=w_sb[:, kc],
                                     start=False, stop=(kc == KC - 1))
                nc.vector.tensor_scalar(out_tile[:, 0:256], px[:, 0:256],
                                        0.0, None, Alu.max)
                nc.scalar.activation(out_tile[:, 256:512], px[:, 256:512],
                                     Act.Relu)

            x1 = cpool.tile([128, D], f16, tag="x1")
            mlp_layer(h[NUM_LAYERS], w0T_sb, b0_sb, x1, "m1")
            x2 = cpool.tile([128, D], f32, tag="x2")
            mlp_layer(x1, w1T_sb, b1_sb, x2, "m2")

            nc.sync.dma_start(out_e[:, 0:256], x2[:, 0:256])
            nc.sync.dma_start(out_e[:, 256:512], x2[:, 256:512])

    nc.compile()
    return nc


def _get_nc():
    if "nc" not in _CACHE:
        _CACHE["nc"] = _build_nc()
    return _CACHE["nc"]


def kernel(gcn_inputs, word_seq_len, adj_matrix, dep_label_matrix,
           w_params, mlp_w0, mlp_b0, mlp_w1, mlp_b1, **_unused):
    from concourse.bass_utils import run_bass_kernel_spmd

    gcn = np.asarray(gcn_inputs, dtype=np.float32)
    adj = np.asarray(adj_matrix, dtype=np.float32)
    lab = np.asarray(dep_label_matrix)
    w = np.asarray(w_params, dtype=np.float32)
    w0 = np.asarray(mlp_w0, dtype=np.float32)
    w1 = np.asarray(mlp_w1, dtype=np.float32)
    b0 = np.asarray(mlp_b0, dtype=np.float32)
    b1 = np.asarray(mlp_b1, dtype=np.float32)

    import ml_dtypes

    # wT[kmod, l, kc, d] = w[l, d, kc*128+kmod]  (shared by all cores).
    # Labels < K8 are stored x64 in BOTH planes (fp8 for layer 1, fp16
    # for layer 2 -- a lossless exponent shift); the kernel scales their
    # aggregated s by 1/64 instead, keeping the fp8 values in e4m3's
    # normal range.
    ws = w.copy()
    ws[:K8] *= 64.0
    wT = ws.transpose(0, 2, 1).reshape(L, KC, 128, D).transpose(2, 0, 1, 3)
    wT = np.ascontiguousarray(wT)
    wT8 = wT[:, :K8].astype(ml_dtypes.float8_e4m3)
    wT = wT.astype(np.float16)
    w0T = np.ascontiguousarray(
        w0.T.reshape(KC, 128, D).transpose(1, 0, 2)).astype(np.float16)
    w1T = np.ascontiguousarray(
        w1.T.reshape(KC, 128, D).transpose(1, 0, 2)).astype(np.float16)
    b0r = b0.reshape(1, D).astype(np.float16)
    b1r = b1.reshape(1, D).astype(np.float16)
    labf = lab.astype(np.float32)

    in_maps = []
    for c in range(NCORES):
        alr = np.stack([adj[c].T, labf[c].T, adj[c]], axis=1)
        in_maps.append({
            "gcn16": gcn[c].astype(np.float16),
            "alr": np.ascontiguousarray(alr),
            "wT": wT,
            "wT8": wT8,
            "w0T": w0T,
            "w1T": w1T,
            "b0": b0r,
            "b1": b1r,
        })

    nc = _get_nc()
    res = run_bass_kernel_spmd(nc, in_maps, list(range(NCORES)))

    out = np.empty((B, N, D), dtype=np.float32)
    for c in range(NCORES):
        out[c] = res.results[c]["out"]
    return out
